# revision 57
# baseline (speedup 1.0000x reference)
"""Trainium2 Bass kernel for the AI4Burgers 3x3-stencil operator.

Reference computation (per batch image, replicate padding):
    Lu = NU*conv3x3(u, w1) - u_vel*conv3x3(u, w2) - u_vel*conv3x3(u, w3)
       = conv3x3(u, NU*w1) - u_vel * conv3x3(u, w2 + w3)

Strategy
- Data-parallel over batch: 16 images across 8 NeuronCores, 2 images/core.
  No cross-core communication.
- Each core's 2 images are processed as one flat [2048, 1024] sheet (the
  images are contiguous in DRAM) cut into 17 row-chunks with H on the SBUF
  partition axis and W on the free axis. The vertical part of the stencil is
  a banded [K, M] stationary matrix on the TensorEngine (per-chunk variants
  bake in the replicate padding at image tops/bottoms, including the interior
  img0|img1 boundary); the horizontal part comes from accumulating matmuls
  whose moving operands are column-shifted views of the same SBUF tile, with
  the W-edge replication folded into two extra N=1 matmuls per bank.
- Compute is bf16 (inputs are pre-rounded on the host), accumulation fp32 in
  PSUM; the pointwise combine (c1 - u_vel*c23) runs on the DVE.
"""

import numpy as np

NU = 0.5
B, H, W = 16, 1024, 1024
NCORES = 8
IMGS_PER_CORE = B // NCORES
FH = H * IMGS_PER_CORE  # flat sheet rows per core

# 17 chunks over the flat sheet: 16 x 126 outputs + 1 x 32.
# (c, K_in_rows, M_out_rows, in_row0, out_row0)
CHUNKS = []
_r = 0
while _r < FH:
    _m = min(126, FH - _r)
    _rin = max(_r - 1, 0)
    _k = min(_r + _m + 1, FH) - _rin
    CHUNKS.append((len(CHUNKS), _k, _m, _rin, _r))
    _r += _m
NCHUNK = len(CHUNKS)
# band type per chunk: 0=top, 1=interior (shared), 2=img-boundary straddle,
# 3=bottom tail. Interior chunks share one band pattern (it only depends on
# the relative offset), so only 4 distinct types exist.
_STRADDLE = next(c for c, K, M, rin, rout in CHUNKS if rout < H <= rout + M)
CTYPE = [
    0 if c == 0 else (2 if c == _STRADDLE else (3 if c == NCHUNK - 1 else 1))
    for c in range(NCHUNK)
]
NTYPE = 4
NSEG = 2 * NTYPE * 3

_cache = {}


def _band(w, K, M, rin, rout):
    """[K, M] vertical-band matrix for one dx column of weights w[3] over the
    flat sheet, with per-image replicate clamping at image top/bottom rows."""
    S = np.zeros((K, M), np.float64)
    for m in range(M):
        row = rout + m  # flat output row
        img = row // H
        lo, hi = img * H, img * H + H - 1
        for dy in range(3):
            src = min(max(row + dy - 1, lo), hi)
            S[src - rin, m] += w[dy]
    return S


def _build_stationaries(w1, w2, w3):
    """[128, NSEG*128] f32: segment s = (conv*NCHUNK + c)*3 + dx."""
    wa = NU * np.asarray(w1, np.float64)[0, 0]
    wb = (np.asarray(w2, np.float64) + np.asarray(w3, np.float64))[0, 0]
    stat = np.zeros((128, NSEG * 128), np.float64)
    done = set()
    for ci, wm in enumerate((wa, wb)):
        for c, K, M, rin, rout in CHUNKS:
            t = CTYPE[c]
            if (ci, t) in done:
                continue
            done.add((ci, t))
            for dx in range(3):
                s = (ci * NTYPE + t) * 3 + dx
                stat[0:K, s * 128 : s * 128 + M] = _band(wm[:, dx], K, M, rin, rout)
    return stat.astype(np.float32)


def _build_program():
    from concourse import bacc, tile, mybir
    from concourse.tile import add_dep_helper

    f32 = mybir.dt.float32
    bf16 = mybir.dt.bfloat16

    nc = bacc.Bacc(None, target_bir_lowering=False, debug=False)
    u_d = nc.dram_tensor("u", [FH, W], bf16, kind="ExternalInput").ap()
    v_d = nc.dram_tensor("uvel", [FH, W], bf16, kind="ExternalInput").ap()
    s_d = nc.dram_tensor("stat", [128, NSEG * 128], bf16, kind="ExternalInput").ap()
    o_d = nc.dram_tensor("out", [FH, W], bf16, kind="ExternalOutput").ap()

    # u tile: data cols [GO, GO+W). GO=16 bf16 elems = 32B so DMA writes land
    # 32B-aligned; width 1056 keeps the partition pitch (2112B) a multiple of
    # 32B and off the power-of-2 stride that causes SBUF bank conflicts.
    GO = 16
    UTW = 1056

    with tile.TileContext(nc) as tc:
        with (
            tc.tile_pool(name="const", bufs=1) as cp,
            tc.tile_pool(name="up", bufs=6) as up,
            tc.tile_pool(name="vp", bufs=6) as vp,
            tc.tile_pool(name="op", bufs=6) as op,
            tc.tile_pool(name="tp", bufs=6) as tp,
            tc.tile_pool(name="pp", bufs=2, space="PSUM") as pp,
        ):
            stat_t = cp.tile([128, NSEG * 128], bf16)
            # quarters q2 (conv1 TOP+MID) and q0 (conv0 TOP+MID) gate the
            # first matmuls: load them on SWDGE ahead of the u stream. The
            # straddle/tail bands (q1, q3) are not read until ~chunk 8, so
            # they ride the slower HWDGE rings.
            _q = NSEG * 32
            for qi, eng in ((2, nc.gpsimd), (0, nc.gpsimd), (1, nc.sync), (3, nc.scalar)):
                eng.dma_start(
                    stat_t[:, qi * _q : (qi + 1) * _q],
                    s_d[:, qi * _q : (qi + 1) * _q],
                )

            # u on SWDGE (one op spreads over all 16 SDMA engines: low
            # latency); u_vel/out alternate on the two HWDGE rings.
            for c, K, M, rin, rout in CHUNKS:
                veng = nc.scalar if c % 2 == 0 else nc.sync
                oeng = nc.sync if c % 2 == 0 else nc.scalar

                ut = up.tile([128, UTW], bf16, tag="ut")
                if c == 0:
                    # first chunk: 4 row-quarter ops so completion semaphores
                    # fire early (a single op's last-engine tail otherwise
                    # gates the first matmul for ~10us)
                    for q0 in range(0, K, 32):
                        q1 = min(q0 + 32, K)
                        ui = nc.gpsimd.dma_start(
                            ut[q0:q1, GO : GO + W], u_d[rin + q0 : rin + q1, :]
                        )
                else:
                    ui = nc.gpsimd.dma_start(
                        ut[0:K, GO : GO + W], u_d[rin : rin + K, :]
                    )

                if c > 0:
                    # W-edge replicate columns filled on ACT: prefetch depth
                    # means the u-DMA sem is long satisfied when ACT gets here
                    nc.scalar.copy(ut[0:K, GO - 1 : GO], ut[0:K, GO : GO + 1])
                    nc.scalar.copy(
                        ut[0:K, GO + W : GO + W + 1],
                        ut[0:K, GO + W - 1 : GO + W],
                    )

                vt = vp.tile([128, W], bf16, tag="vt")
                vi = veng.dma_start(vt[0:M, :], v_d[rout : rout + M, :])
                # u_vel load waits for this chunk's u load: the HWDGE packet
                # burst must not hog SDMA engines while u tiles (the critical
                # path) are draining
                add_dep_helper(vi.ins, ui.ins, sync=True, reason="vt after ut")

                ot = op.tile([128, W], bf16, tag="ot")
                # one 2-bank PSUM tile per conv: halves h=0/1 are the banks.
                # conv23 (pB) runs first so the DVE multiply overlaps conv1's
                # matmuls instead of waiting for all of them.
                pA = pp.tile([128, 1024], f32, tag="pA", name="pA")
                pB = pp.tile([128, 1024], f32, tag="pB", name="pB")
                tt = tp.tile([128, W], bf16, tag="tt")
                aSB = tp.tile([128, W], bf16, tag="aSB", name="aSB")
                # W-edge replicate padding is folded into extra N=1 matmuls:
                #   h=0: dx0 covers out cols 1..511; out col 0 += w[:,0]*u[:,0]
                #   h=1: dx2 covers out cols 0..510; col 1023 += w[:,2]*u[:,W-1]
                # Stationaries are the full 128 columns (zero rows beyond M)
                # so FWL stays eligible (NumWeights==128).
                for conv, pt in ((1, pB), (0, pA)):
                    segb = ((conv * NTYPE + CTYPE[c]) * 3) * 128
                    s0 = stat_t[0:K, segb : segb + 128]
                    s1 = stat_t[0:K, segb + 128 : segb + 256]
                    s2 = stat_t[0:K, segb + 256 : segb + 384]
                    if c == 0:
                        # first chunk avoids the ACT edge fills entirely
                        # (N=1 edge matmuls instead) so nothing gates the
                        # pipeline start
                        nc.tensor.matmul(
                            pt[0:128, 1:512], s0, ut[0:K, GO : GO + 511],
                            start=True, stop=False,
                        )
                        nc.tensor.matmul(
                            pt[0:128, 0:1], s0, ut[0:K, GO : GO + 1],
                            start=False, stop=False,
                        )
                        nc.tensor.matmul(
                            pt[0:128, 0:512], s1, ut[0:K, GO : GO + 512],
                            start=False, stop=False,
                        )
                        nc.tensor.matmul(
                            pt[0:128, 0:512], s2, ut[0:K, GO + 1 : GO + 513],
                            start=False, stop=True,
                        )
                        nc.tensor.matmul(
                            pt[0:128, 512:1024], s0,
                            ut[0:K, GO + 511 : GO + 1023],
                            start=True, stop=False,
                        )
                        nc.tensor.matmul(
                            pt[0:128, 512:1024], s1,
                            ut[0:K, GO + 512 : GO + 1024],
                            start=False, stop=False,
                        )
                        nc.tensor.matmul(
                            pt[0:128, 512:1023], s2,
                            ut[0:K, GO + 513 : GO + 1024],
                            start=False, stop=False,
                        )
                        nc.tensor.matmul(
                            pt[0:128, 1023:1024], s2,
                            ut[0:K, GO + W - 1 : GO + W],
                            start=False, stop=True,
                        )
                    else:
                        # 3 clean N=512 matmuls per bank; s1 (needs no edge
                        # columns) opens each group so the PE can start while
                        # the ACT edge fills land
                        for hh in range(2):
                            ob = 512 * hh
                            nc.tensor.matmul(
                                pt[0:128, ob : ob + 512],
                                s1,
                                ut[0:K, GO + ob : GO + ob + 512],
                                start=True, stop=False,
                            )
                            nc.tensor.matmul(
                                pt[0:128, ob : ob + 512],
                                s0,
                                ut[0:K, GO - 1 + ob : GO - 1 + ob + 512],
                                start=False, stop=False,
                            )
                            nc.tensor.matmul(
                                pt[0:128, ob : ob + 512],
                                s2,
                                ut[0:K, GO + 1 + ob : GO + 1 + ob + 512],
                                start=False, stop=True,
                            )
                    if conv == 1:
                        if c == NCHUNK - 1:
                            # last chunk: per-bank combine overlaps the DVE
                            # work with the remaining matmuls (shorter tail)
                            for hh in range(2):
                                sl = slice(512 * hh, 512 * hh + 512)
                                nc.vector.tensor_mul(
                                    tt[0:M, sl], vt[0:M, sl], pB[0:M, sl]
                                )
                        else:
                            nc.vector.tensor_mul(
                                tt[0:M, :], vt[0:M, :], pB[0:M, :]
                            )
                # ACT evacuates pA (it reads PSUM fast and is otherwise
                # idle); the DVE subtract then runs SBUF-only at 2x rate
                nc.scalar.copy(aSB[0:M, :], pA[0:M, :])
                nc.vector.tensor_sub(ot[0:M, :], aSB[0:M, :], tt[0:M, :])
                oeng.dma_start(o_d[rout : rout + M, :], ot[0:M, :])

    nc.compile()
    return nc


def _get_program():
    if "nc" not in _cache:
        _cache["nc"] = _build_program()
    return _cache["nc"]


def _make_in_maps(u, u_vel, w1, w2, w3):
    import ml_dtypes

    bf = ml_dtypes.bfloat16
    u = np.ascontiguousarray(np.asarray(u, np.float32).reshape(B, H, W).astype(bf))
    v = np.ascontiguousarray(np.asarray(u_vel, np.float32).reshape(B, H, W).astype(bf))
    stat = _build_stationaries(w1, w2, w3).astype(bf)
    n = IMGS_PER_CORE
    return [
        {
            "u": u[i * n : (i + 1) * n].reshape(FH, W),
            "uvel": v[i * n : (i + 1) * n].reshape(FH, W),
            "stat": stat,
        }
        for i in range(NCORES)
    ]


def kernel(u, u_vel, w1, w2, w3):
    from concourse.bass_utils import run_bass_kernel_spmd

    nc = _get_program()
    in_maps = _make_in_maps(u, u_vel, w1, w2, w3)
    res = run_bass_kernel_spmd(nc, in_maps, core_ids=list(range(NCORES)))
    out = np.empty((B, 1, H, W), np.float32)
    n = IMGS_PER_CORE
    for i in range(NCORES):
        out[i * n : (i + 1) * n, 0] = (
            res.results[i]["out"].astype(np.float32).reshape(n, H, W)
        )
    return out

